# revision 11
# baseline (speedup 1.0000x reference)
"""Trainium2 Bass kernel for the Bolt 64-QAM demapper MLP forward pass.

Problem: llr = (relu(relu(z @ W1 + b1) @ W2 + b2) @ W3 + b3).reshape(B, S*6)
  z [4096, 512, 3] f32, W1 [3,128], W2 [128,128], W3 [128,6].

Strategy: pure data parallel over 8 NeuronCores (batch split), 262144 rows
per core, 2048-row tiles (4 chunks of 512 rows).

The on-chip bottleneck is PSUM evacuation: only ACT and DVE can read PSUM
(1 elem/cycle/partition), and h1+h2+out must each cross PSUM->SBUF once.
So the kernel is organized to keep ACT and DVE 100% busy on evacuation and
nothing else:

  * z is pre-transposed ON THE HOST into the exact moving-operand layout
    (bf16, feature-major, 4th feature = 1.0 so b1 folds into W1 as a K=4
    matmul) -- no on-chip transpose/expand/memset at all.
  * the output is stored feature-major bf16 and un-transposed ON THE HOST
    (+b3, f32 cast) -- no on-chip output transpose/pack.
  * per tile: L1 = 4 row-packed K=4 matmuls -> h1 PSUM as two [128,1024]
    halves; ACT evacuates each half (fused relu, f32->bf16); L2 = 4 K=128
    matmuls; DVE evacuates the four h2 chunks (fused +b2, relu, bf16);
    L3 = 4 col-packed matmuls (W3 zero-padded to 32) into one PSUM bank;
    ACT copy-evacuates it (bf16) into a per-quad staging buffer that DMAs
    out 24 valid partitions per strip.
  * PSUM: h1 2x2 banks + h2 2x1 + out 2x1 = all 8 banks.
  * every engine's instruction stream is explicitly chained (ordering
    deps) in a software-pipelined order so the scheduler cannot interleave
    packed matmul groups or delay the evacuations that gate the pipeline.
"""
import os
import numpy as np
from contextlib import ExitStack

import concourse.bacc as bacc
import concourse.mybir as mybir
import concourse.tile as tile
from concourse import bass_utils
from bass_rust import add_dep_helper

F32 = mybir.dt.float32
BF16 = mybir.dt.bfloat16
AF = mybir.ActivationFunctionType
ALU = mybir.AluOpType

N_CORES = 8
B, S, H, NB = 4096, 512, 128, 6
ROWS_TOTAL = B * S                    # 2097152
ROWS_CORE = ROWS_TOTAL // N_CORES     # 262144
TROWS = 2048                          # rows per tile
NT = ROWS_CORE // TROWS               # 128 tiles
NQ = NT // 4                          # 32 quads
PREFETCH_Q = 2                        # z quad prefetch depth

LAST_RESULTS = None  # stashed BassKernelResults for test harness inspection


def _build_nc():
    nc = bacc.Bacc("TRN2", target_bir_lowering=False, debug=False, num_devices=N_CORES)
    # z4 rows: (Q*4 + a)*4 + u ; cols: tq*512 + j ; value = feat u of row
    # R = (Q*4+tq)*2048 + a*512 + j (u=3 -> 1.0, folds b1 into W1)
    z4_d = nc.dram_tensor("z4", [NQ * 4 * 4, 2048], BF16, kind="ExternalInput")
    w1rep_d = nc.dram_tensor("w1rep", [128, H], BF16, kind="ExternalInput")
    w2_d = nc.dram_tensor("w2", [H, H], BF16, kind="ExternalInput")
    w3_d = nc.dram_tensor("w3", [H, 32], BF16, kind="ExternalInput")
    b2_d = nc.dram_tensor("b2", [H, 1], F32, kind="ExternalInput")
    # out4 rows: (Q*4 + a)*6 + u ; cols: tq*512 + j ; llr (pre-b3) bf16
    out4_d = nc.dram_tensor("out4", [NQ * 4 * NB, 2048], BF16, kind="ExternalOutput")

    with tile.TileContext(nc) as tc, ExitStack() as ctx:
        const = ctx.enter_context(tc.tile_pool(name="const", bufs=1))
        zqp = ctx.enter_context(tc.tile_pool(name="zqp", bufs=3))
        h1p = ctx.enter_context(tc.tile_pool(name="h1p", bufs=3))
        h2p = ctx.enter_context(tc.tile_pool(name="h2p", bufs=10))
        oqp = ctx.enter_context(tc.tile_pool(name="oqp", bufs=3))
        ps_h1 = ctx.enter_context(tc.tile_pool(name="ps_h1", bufs=1, space="PSUM"))
        ps_h2 = ctx.enter_context(tc.tile_pool(name="ps_h2", bufs=3, space="PSUM"))
        ps_o = ctx.enter_context(tc.tile_pool(name="ps_o", bufs=1, space="PSUM"))

        w1rep = const.tile([128, H], BF16)
        nc.sync.dma_start(w1rep[:], w1rep_d.ap())
        w2sb = const.tile([H, H], BF16)
        nc.sync.dma_start(w2sb[:], w2_d.ap())
        w3sb = const.tile([H, 32], BF16)
        nc.sync.dma_start(w3sb[:], w3_d.ap())
        b2sb = const.tile([H, 1], F32)
        nc.sync.dma_start(b2sb[:], b2_d.ap())

        z4_v = z4_d.ap().rearrange("(q a u) c -> q a u c", q=NQ, a=4)
        out4_v = out4_d.ap().rearrange("(q a u) c -> q a u c", q=NQ, a=4)

        # --- per-engine explicit ordering chains -------------------------
        last = {"pe": None, "act": None, "dve": None}

        def chain(eng, inst, why):
            if last[eng] is not None:
                add_dep_helper(inst.ins, last[eng].ins, False, why)
            last[eng] = inst
            return inst

        def mm(*args, **kw):
            return chain("pe", nc.tensor.matmul(*args, **kw), "pe order")

        def act(fn, *args, **kw):
            return chain("act", fn(*args, **kw), "act order")

        def dve(fn, *args, **kw):
            return chain("dve", fn(*args, **kw), "dve order")

        # --- state carried across pipeline stages ------------------------
        zqs = {}      # quad -> z staging tile [128, 2048] bf16
        h1ps = {}     # (t, half) -> PSUM [128, 1024] f32
        h1sb = {}     # (t, half) -> SBUF [128, 1024] bf16
        h2ps = {}     # (t, c) -> PSUM [128, 512] f32
        h2sb = {}     # (t, c) -> SBUF [128, 512] bf16
        ops_ = {}     # t -> out PSUM [128, 512] f32
        outqs = {}    # quad -> out staging tile [128, 2048] bf16

        def load_quad(q):
            zq = zqp.tile([128, 2048], BF16, tag="zq")
            for a in range(4):
                nc.sync.dma_start(zq[32 * a : 32 * a + 4, :], z4_v[q][a])
            zqs[q] = zq

        def l1(t):
            q, tq = divmod(t, 4)
            h1_ps = ps_h1.tile([128, 2048], F32, tag="h1ps", name="h1ps")
            for a in range(4):
                mm(
                    h1_ps[:, a * 512 : (a + 1) * 512],
                    w1rep[32 * a : 32 * a + 4, :],
                    zqs[q][32 * a : 32 * a + 4, tq * 512 : (tq + 1) * 512],
                    tile_position=(32 * a, 0),
                )
            h1ps[t] = h1_ps

        def evac_h1(t):
            h1_ps = h1ps.pop(t)
            h1_sb = h1p.tile([128, 2048], BF16, tag="h1sb", name="h1sb")
            act(nc.scalar.activation, h1_sb[:], h1_ps[:], AF.Relu)
            h1sb[t] = h1_sb

        def l2_chunk(t, c):
            h1_sb = h1sb[t]
            h2_ps = ps_h2.tile([128, 512], F32, tag="h2ps", name="h2ps")
            mm(h2_ps[:], w2sb[:], h1_sb[:, c * 512 : (c + 1) * 512])
            h2ps[(t, c)] = h2_ps
            if c == 3:
                h1sb.pop(t)

        def evac_h2(t, c):
            h2_ps = h2ps.pop((t, c))
            h2_sb = h2p.tile([128, 512], BF16, tag="h2sb", name="h2sb")
            dve(
                nc.vector.tensor_scalar,
                h2_sb[:], h2_ps[:], b2sb[:], 0.0, op0=ALU.add, op1=ALU.max,
            )
            h2sb[(t, c)] = h2_sb

        def l3(t):
            out_ps = ps_o.tile([128, 512], F32, tag="ops", name="ops")
            for a in range(4):
                mm(
                    out_ps[32 * a : 32 * a + 32, :],
                    w3sb[:],
                    h2sb.pop((t, a))[:],
                    tile_position=(0, 32 * a),
                )
            ops_[t] = out_ps

        def evac_out(t):
            q, tq = divmod(t, 4)
            if tq == 0:
                outqs[q] = oqp.tile([128, 2048], BF16, tag="outq", name="outq")
            out_ps = ops_.pop(t)
            act(
                nc.scalar.activation,
                outqs[q][:, tq * 512 : (tq + 1) * 512], out_ps[:], AF.Copy,
            )
            if tq == 3:
                oq = outqs.pop(q)
                for a in range(4):
                    nc.sync.dma_start(out4_v[q][a], oq[32 * a : 32 * a + NB, :])

        # --- software-pipelined emission ---------------------------------
        for q in range(min(PREFETCH_Q + 1, NQ)):
            load_quad(q)

        for s in range(NT + 3):
            if s < NT and s % 4 == 0:
                qn = s // 4 + PREFETCH_Q + 1
                if qn < NQ:
                    load_quad(qn)
            if s < NT:
                l1(s)
                evac_h1(s)
            if 1 <= s <= NT:
                for c in range(4):
                    l2_chunk(s - 1, c)
                    evac_h2(s - 1, c)
            if 3 <= s <= NT + 2:
                evac_out(s - 3)
            if 2 <= s <= NT + 1:
                l3(s - 2)

    nc.compile()
    return nc


def _prep_core_z(z_core_rows: np.ndarray, npbf16) -> np.ndarray:
    # [262144, 3] f32 -> [(Q a u), 2048] bf16 with u=3 a ones-row
    zr = z_core_rows.reshape(NQ, 4, 4, 512, 3)          # (Q, tq, a, j, u)
    zr = zr.transpose(0, 2, 4, 1, 3)                    # (Q, a, u, tq, j)
    out = np.ones((NQ, 4, 4, 4, 512), dtype=np.float32)
    out[:, :, :3] = zr
    return np.ascontiguousarray(out.astype(npbf16).reshape(NQ * 16, 2048))


def kernel(z, W1, b1, W2, b2, W3, b3):
    global LAST_RESULTS
    z = np.asarray(z, dtype=np.float32)
    W1 = np.asarray(W1, dtype=np.float32)
    b1 = np.asarray(b1, dtype=np.float32)
    W2 = np.asarray(W2, dtype=np.float32)
    b2 = np.asarray(b2, dtype=np.float32)
    W3 = np.asarray(W3, dtype=np.float32)
    b3 = np.asarray(b3, dtype=np.float32)
    npbf16 = mybir.dt.np(BF16)

    # host-side weight prep (tiny): fold b1 into W1 as 4th input feature
    w1p = np.concatenate([W1, b1.reshape(1, H)], axis=0)  # [4, 128]
    w1rep = np.zeros((128, H), npbf16)
    for a in range(4):
        w1rep[32 * a : 32 * a + 4] = w1p.astype(npbf16)
    w3pad = np.zeros((H, 32), npbf16)
    w3pad[:, :NB] = W3.astype(npbf16)

    z_rows = np.ascontiguousarray(z).reshape(ROWS_TOTAL, 3)
    shards = np.split(z_rows, N_CORES, axis=0)

    common = {
        "w1rep": w1rep,
        "w2": np.ascontiguousarray(W2.astype(npbf16)),
        "w3": w3pad,
        "b2": np.ascontiguousarray(b2.reshape(H, 1)),
    }
    in_maps = [dict(common, z4=_prep_core_z(s, npbf16)) for s in shards]

    nc = _build_nc()
    res = bass_utils.run_bass_kernel_spmd(
        nc,
        in_maps,
        core_ids=list(range(N_CORES)),
        trace=bool(os.environ.get("KBENCH_TRACE")),
    )
    LAST_RESULTS = res

    # host-side un-transpose + b3 + f32 cast
    outs = []
    for i in range(N_CORES):
        o4 = res.results[i]["out4"].astype(np.float32)
        o4 = o4.reshape(NQ, 4, NB, 4, 512)              # (Q, a, u, tq, j)
        o4 = o4.transpose(0, 3, 1, 4, 2)                # (Q, tq, a, j, u)
        outs.append(o4.reshape(ROWS_CORE, NB))
    full = np.concatenate(outs, axis=0) + b3.reshape(1, NB)
    return full.reshape(B, S * NB).astype(np.float32)


# revision 14
# speedup vs baseline: 1.0539x; 1.0539x over previous
"""Trainium2 Bass kernel for the Bolt 64-QAM demapper MLP forward pass.

Problem: llr = (relu(relu(z @ W1 + b1) @ W2 + b2) @ W3 + b3).reshape(B, S*6)
  z [4096, 512, 3] f32, W1 [3,128], W2 [128,128], W3 [128,6].

Strategy: pure data parallel over 8 NeuronCores (batch split), 262144 rows
per core, 2048-row tiles (4 chunks of 512 rows).

The on-chip bottleneck is PSUM evacuation: only ACT and DVE can read PSUM
(1 elem/cycle/partition), and h1+h2+out must each cross PSUM->SBUF once.
So the kernel is organized to keep ACT and DVE 100% busy on evacuation and
nothing else:

  * z is pre-transposed ON THE HOST into the exact moving-operand layout
    (bf16, feature-major, 4th feature = 1.0 so b1 folds into W1 as a K=4
    matmul) -- no on-chip transpose/expand/memset at all.
  * the output is stored feature-major bf16 and un-transposed ON THE HOST
    (+b3, f32 cast) -- no on-chip output transpose/pack.
  * per tile: L1 = 4 row-packed K=4 matmuls -> h1 PSUM as two [128,1024]
    halves; ACT evacuates each half (fused relu, f32->bf16); L2 = 4 K=128
    matmuls; DVE evacuates the four h2 chunks (fused +b2, relu, bf16);
    L3 = 4 col-packed matmuls (W3 zero-padded to 32) into one PSUM bank;
    ACT copy-evacuates it (bf16) into a per-quad staging buffer that DMAs
    out 24 valid partitions per strip.
  * PSUM: h1 2x2 banks + h2 2x1 + out 2x1 = all 8 banks.
  * every engine's instruction stream is explicitly chained (ordering
    deps) in a software-pipelined order so the scheduler cannot interleave
    packed matmul groups or delay the evacuations that gate the pipeline.
"""
import os
import numpy as np
from contextlib import ExitStack

import concourse.bacc as bacc
import concourse.mybir as mybir
import concourse.tile as tile
from concourse import bass_utils
from bass_rust import add_dep_helper

F32 = mybir.dt.float32
BF16 = mybir.dt.bfloat16
AF = mybir.ActivationFunctionType
ALU = mybir.AluOpType

N_CORES = 8
B, S, H, NB = 4096, 512, 128, 6
ROWS_TOTAL = B * S                    # 2097152
ROWS_CORE = ROWS_TOTAL // N_CORES     # 262144
TROWS = 2048                          # rows per tile
NT = ROWS_CORE // TROWS               # 128 tiles
NQ = NT // 4                          # 32 quads
PREFETCH_Q = 2                        # z quad prefetch depth

LAST_RESULTS = None  # stashed BassKernelResults for test harness inspection


def _build_nc():
    nc = bacc.Bacc("TRN2", target_bir_lowering=False, debug=False, num_devices=N_CORES)
    # z4 rows: (Q*4 + a)*4 + u ; cols: tq*512 + j ; value = feat u of row
    # R = (Q*4+tq)*2048 + a*512 + j (u=3 -> 1.0, folds b1 into W1)
    z4_d = nc.dram_tensor("z4", [NQ * 4 * 4, 2048], BF16, kind="ExternalInput")
    w1rep_d = nc.dram_tensor("w1rep", [128, H], BF16, kind="ExternalInput")
    w2_d = nc.dram_tensor("w2", [H, H], BF16, kind="ExternalInput")
    w3_d = nc.dram_tensor("w3", [H, 32], BF16, kind="ExternalInput")
    b2_d = nc.dram_tensor("b2", [H, 1], F32, kind="ExternalInput")
    # out4 rows: (Q*4 + a)*6 + u ; cols: tq*512 + j ; llr (pre-b3) bf16
    out4_d = nc.dram_tensor("out4", [NQ * 4 * NB, 2048], BF16, kind="ExternalOutput")

    with tile.TileContext(nc) as tc, ExitStack() as ctx:
        const = ctx.enter_context(tc.tile_pool(name="const", bufs=1))
        zqp = ctx.enter_context(tc.tile_pool(name="zqp", bufs=3))
        h1p = ctx.enter_context(tc.tile_pool(name="h1p", bufs=3))
        h2p = ctx.enter_context(tc.tile_pool(name="h2p", bufs=14))
        oqp = ctx.enter_context(tc.tile_pool(name="oqp", bufs=3))
        ps_h1 = ctx.enter_context(tc.tile_pool(name="ps_h1", bufs=1, space="PSUM"))
        ps_h2 = ctx.enter_context(tc.tile_pool(name="ps_h2", bufs=3, space="PSUM"))
        ps_o = ctx.enter_context(tc.tile_pool(name="ps_o", bufs=1, space="PSUM"))

        w1rep = const.tile([128, H], BF16)
        nc.sync.dma_start(w1rep[:], w1rep_d.ap())
        w2sb = const.tile([H, H], BF16)
        nc.sync.dma_start(w2sb[:], w2_d.ap())
        w3sb = const.tile([H, 32], BF16)
        nc.sync.dma_start(w3sb[:], w3_d.ap())
        b2sb = const.tile([H, 1], F32)
        nc.sync.dma_start(b2sb[:], b2_d.ap())

        z4_v = z4_d.ap().rearrange("(q a u) c -> q a u c", q=NQ, a=4)
        out4_v = out4_d.ap().rearrange("(q a u) c -> q a u c", q=NQ, a=4)

        # --- per-engine explicit ordering chains -------------------------
        last = {"pe": None, "act": None, "dve": None}

        def chain(eng, inst, why):
            if last[eng] is not None:
                add_dep_helper(inst.ins, last[eng].ins, False, why)
            last[eng] = inst
            return inst

        def mm(*args, **kw):
            return chain("pe", nc.tensor.matmul(*args, **kw), "pe order")

        def act(fn, *args, **kw):
            return chain("act", fn(*args, **kw), "act order")

        def dve(fn, *args, **kw):
            return chain("dve", fn(*args, **kw), "dve order")

        # --- state carried across pipeline stages ------------------------
        zqs = {}      # quad -> z staging tile [128, 2048] bf16
        h1ps = {}     # (t, half) -> PSUM [128, 1024] f32
        h1sb = {}     # (t, half) -> SBUF [128, 1024] bf16
        h2ps = {}     # (t, c) -> PSUM [128, 512] f32
        h2sb = {}     # (t, c) -> SBUF [128, 512] bf16
        ops_ = {}     # t -> out PSUM [128, 512] f32
        outqs = {}    # quad -> out staging tile [128, 2048] bf16

        def load_quad(q):
            zq = zqp.tile([128, 2048], BF16, tag="zq")
            for a in range(4):
                nc.sync.dma_start(zq[32 * a : 32 * a + 4, :], z4_v[q][a])
            zqs[q] = zq

        def l1(t):
            q, tq = divmod(t, 4)
            h1_ps = ps_h1.tile([128, 2048], F32, tag="h1ps", name="h1ps")
            for a in range(4):
                mm(
                    h1_ps[:, a * 512 : (a + 1) * 512],
                    w1rep[32 * a : 32 * a + 4, :],
                    zqs[q][32 * a : 32 * a + 4, tq * 512 : (tq + 1) * 512],
                    tile_position=(32 * a, 0),
                )
            h1ps[t] = h1_ps

        def evac_h1(t):
            h1_ps = h1ps.pop(t)
            h1_sb = h1p.tile([128, 2048], BF16, tag="h1sb", name="h1sb")
            act(nc.scalar.activation, h1_sb[:], h1_ps[:], AF.Relu)
            h1sb[t] = h1_sb

        def l2_chunk(t, c):
            h1_sb = h1sb[t]
            h2_ps = ps_h2.tile([128, 512], F32, tag="h2ps", name="h2ps")
            mm(h2_ps[:], w2sb[:], h1_sb[:, c * 512 : (c + 1) * 512])
            h2ps[(t, c)] = h2_ps
            if c == 3:
                h1sb.pop(t)

        def evac_h2(t, c):
            h2_ps = h2ps.pop((t, c))
            h2_sb = h2p.tile([128, 512], BF16, tag="h2sb", name="h2sb")
            dve(
                nc.vector.tensor_scalar,
                h2_sb[:], h2_ps[:], b2sb[:], 0.0, op0=ALU.add, op1=ALU.max,
            )
            h2sb[(t, c)] = h2_sb

        def l3(t):
            out_ps = ps_o.tile([128, 512], F32, tag="ops", name="ops")
            for a in range(4):
                mm(
                    out_ps[32 * a : 32 * a + 32, :],
                    w3sb[:],
                    h2sb.pop((t, a))[:],
                    tile_position=(0, 32 * a),
                )
            ops_[t] = out_ps

        def evac_out(t):
            q, tq = divmod(t, 4)
            if tq == 0:
                outqs[q] = oqp.tile([128, 2048], BF16, tag="outq", name="outq")
            out_ps = ops_.pop(t)
            act(
                nc.scalar.activation,
                outqs[q][:, tq * 512 : (tq + 1) * 512], out_ps[:], AF.Copy,
            )
            if tq == 3:
                oq = outqs.pop(q)
                for a in range(4):
                    nc.sync.dma_start(out4_v[q][a], oq[32 * a : 32 * a + NB, :])

        # --- software-pipelined emission ---------------------------------
        for q in range(min(PREFETCH_Q + 1, NQ)):
            load_quad(q)

        for s in range(NT + 3):
            if s < NT and s % 4 == 0:
                qn = s // 4 + PREFETCH_Q + 1
                if qn < NQ:
                    load_quad(qn)
            if s < NT:
                l1(s)
                evac_h1(s)
            if 3 <= s <= NT + 2:
                l3(s - 3)
                evac_out(s - 3)
            if 1 <= s <= NT:
                for c in range(4):
                    l2_chunk(s - 1, c)
                    evac_h2(s - 1, c)

    nc.compile()
    return nc


def _prep_core_z(z_core_rows: np.ndarray, npbf16) -> np.ndarray:
    # [262144, 3] f32 -> [(Q a u), 2048] bf16 with u=3 a ones-row
    zr = z_core_rows.reshape(NQ, 4, 4, 512, 3)          # (Q, tq, a, j, u)
    zr = zr.transpose(0, 2, 4, 1, 3)                    # (Q, a, u, tq, j)
    out = np.ones((NQ, 4, 4, 4, 512), dtype=np.float32)
    out[:, :, :3] = zr
    return np.ascontiguousarray(out.astype(npbf16).reshape(NQ * 16, 2048))


def kernel(z, W1, b1, W2, b2, W3, b3):
    global LAST_RESULTS
    z = np.asarray(z, dtype=np.float32)
    W1 = np.asarray(W1, dtype=np.float32)
    b1 = np.asarray(b1, dtype=np.float32)
    W2 = np.asarray(W2, dtype=np.float32)
    b2 = np.asarray(b2, dtype=np.float32)
    W3 = np.asarray(W3, dtype=np.float32)
    b3 = np.asarray(b3, dtype=np.float32)
    npbf16 = mybir.dt.np(BF16)

    # host-side weight prep (tiny): fold b1 into W1 as 4th input feature
    w1p = np.concatenate([W1, b1.reshape(1, H)], axis=0)  # [4, 128]
    w1rep = np.zeros((128, H), npbf16)
    for a in range(4):
        w1rep[32 * a : 32 * a + 4] = w1p.astype(npbf16)
    w3pad = np.zeros((H, 32), npbf16)
    w3pad[:, :NB] = W3.astype(npbf16)

    z_rows = np.ascontiguousarray(z).reshape(ROWS_TOTAL, 3)
    shards = np.split(z_rows, N_CORES, axis=0)

    common = {
        "w1rep": w1rep,
        "w2": np.ascontiguousarray(W2.astype(npbf16)),
        "w3": w3pad,
        "b2": np.ascontiguousarray(b2.reshape(H, 1)),
    }
    in_maps = [dict(common, z4=_prep_core_z(s, npbf16)) for s in shards]

    nc = _build_nc()
    res = bass_utils.run_bass_kernel_spmd(
        nc,
        in_maps,
        core_ids=list(range(N_CORES)),
        trace=bool(os.environ.get("KBENCH_TRACE")),
    )
    LAST_RESULTS = res

    # host-side un-transpose + b3 + f32 cast
    outs = []
    for i in range(N_CORES):
        o4 = res.results[i]["out4"].astype(np.float32)
        o4 = o4.reshape(NQ, 4, NB, 4, 512)              # (Q, a, u, tq, j)
        o4 = o4.transpose(0, 3, 1, 4, 2)                # (Q, tq, a, j, u)
        outs.append(o4.reshape(ROWS_CORE, NB))
    full = np.concatenate(outs, axis=0) + b3.reshape(1, NB)
    return full.reshape(B, S * NB).astype(np.float32)


# revision 18
# speedup vs baseline: 1.2627x; 1.1981x over previous
"""Trainium2 Bass kernel for the Bolt 64-QAM demapper MLP forward pass.

Problem: llr = (relu(relu(z @ W1 + b1) @ W2 + b2) @ W3 + b3).reshape(B, S*6)
  z [4096, 512, 3] f32, W1 [3,128], W2 [128,128], W3 [128,6].

Strategy: pure data parallel over 8 NeuronCores (batch split), 262144 rows
per core, 2048-row tiles (4 chunks of 512 rows).

The on-chip bottleneck is PSUM evacuation: only ACT and DVE can read PSUM
(1 elem/cycle/partition), and h1+h2+out must each cross PSUM->SBUF once.
So the kernel is organized to keep ACT and DVE 100% busy on evacuation and
nothing else:

  * z is pre-transposed ON THE HOST into the exact moving-operand layout
    (bf16, feature-major, 4th feature = 1.0 so b1 folds into W1 as a K=4
    matmul) -- no on-chip transpose/expand/memset at all.
  * the output is stored feature-major bf16 and un-transposed ON THE HOST
    (+b3, f32 cast) -- no on-chip output transpose/pack.
  * per tile: L1 = 4 row-packed K=4 matmuls -> h1 PSUM as two [128,1024]
    halves; ACT evacuates each half (fused relu, f32->bf16); L2 = 4 K=128
    matmuls; DVE evacuates the four h2 chunks (fused +b2, relu, bf16);
    L3 = 4 col-packed matmuls (W3 zero-padded to 32) into one PSUM bank;
    ACT copy-evacuates it (bf16) into a per-quad staging buffer that DMAs
    out 24 valid partitions per strip.
  * PSUM: h1 2x2 banks + h2 2x1 + out 2x1 = all 8 banks.
  * every engine's instruction stream is explicitly chained (ordering
    deps) in a software-pipelined order so the scheduler cannot interleave
    packed matmul groups or delay the evacuations that gate the pipeline.
"""
import os
import numpy as np
from contextlib import ExitStack

import concourse.bacc as bacc
import concourse.mybir as mybir
import concourse.tile as tile
from concourse import bass_utils
from bass_rust import add_dep_helper

F32 = mybir.dt.float32
BF16 = mybir.dt.bfloat16
AF = mybir.ActivationFunctionType
ALU = mybir.AluOpType

N_CORES = 8
B, S, H, NB = 4096, 512, 128, 6
ROWS_TOTAL = B * S                    # 2097152
ROWS_CORE = ROWS_TOTAL // N_CORES     # 262144
TROWS = 2048                          # rows per tile
NT = ROWS_CORE // TROWS               # 128 tiles
NQ = NT // 4                          # 32 quads
PREFETCH_Q = 2                        # z quad prefetch depth

LAST_RESULTS = None  # stashed BassKernelResults for test harness inspection


def _build_nc():
    nc = bacc.Bacc("TRN2", target_bir_lowering=False, debug=False, num_devices=N_CORES)
    # z4 rows: (Q*4 + a)*4 + u ; cols: tq*512 + j ; value = feat u of row
    # R = (Q*4+tq)*2048 + a*512 + j (u=3 -> 1.0, folds b1 into W1)
    z4_d = nc.dram_tensor("z4", [NQ * 4 * 4, 2048], BF16, kind="ExternalInput")
    w1rep_d = nc.dram_tensor("w1rep", [128, H], BF16, kind="ExternalInput")
    w2_d = nc.dram_tensor("w2", [H, H], BF16, kind="ExternalInput")
    w3_d = nc.dram_tensor("w3", [H, 32], BF16, kind="ExternalInput")
    b2_d = nc.dram_tensor("b2", [H, 1], F32, kind="ExternalInput")
    # out4 rows: (Q*4 + a)*6 + u ; cols: tq*512 + j ; llr (pre-b3) bf16
    out4_d = nc.dram_tensor("out4", [NQ * 4 * NB, 2048], BF16, kind="ExternalOutput")

    with tile.TileContext(nc) as tc, ExitStack() as ctx:
        const = ctx.enter_context(tc.tile_pool(name="const", bufs=1))
        zqp = ctx.enter_context(tc.tile_pool(name="zqp", bufs=3))
        h1p = ctx.enter_context(tc.tile_pool(name="h1p", bufs=3))
        h2p = ctx.enter_context(tc.tile_pool(name="h2p", bufs=10))
        oqp = ctx.enter_context(tc.tile_pool(name="oqp", bufs=2))
        ps_h1 = ctx.enter_context(tc.tile_pool(name="ps_h1", bufs=1, space="PSUM"))
        ps_h2 = ctx.enter_context(tc.tile_pool(name="ps_h2", bufs=3, space="PSUM"))
        ps_o = ctx.enter_context(tc.tile_pool(name="ps_o", bufs=1, space="PSUM"))

        w1rep = const.tile([128, H], BF16)
        nc.sync.dma_start(w1rep[:], w1rep_d.ap())
        w2sb = const.tile([H, H], BF16)
        nc.sync.dma_start(w2sb[:], w2_d.ap())
        w3sb = const.tile([H, 32], BF16)
        nc.sync.dma_start(w3sb[:], w3_d.ap())
        b2sb = const.tile([H, 1], F32)
        nc.sync.dma_start(b2sb[:], b2_d.ap())

        z4_v = z4_d.ap().rearrange("(q a u) c -> q a u c", q=NQ, a=4)
        out4_v = out4_d.ap().rearrange("(q a u) c -> q a u c", q=NQ, a=4)

        # --- per-engine explicit ordering chains -------------------------
        last = {"pe": None, "act": None, "dve": None}

        def chain(eng, inst, why):
            if last[eng] is not None:
                add_dep_helper(inst.ins, last[eng].ins, False, why)
            last[eng] = inst
            return inst

        def mm(*args, **kw):
            return chain("pe", nc.tensor.matmul(*args, **kw), "pe order")

        def act(fn, *args, **kw):
            return chain("act", fn(*args, **kw), "act order")

        def dve(fn, *args, **kw):
            return chain("dve", fn(*args, **kw), "dve order")

        # --- state carried across pipeline stages ------------------------
        zqs = {}      # quad -> z staging tile [128, 2048] bf16
        h1ps = {}     # (t, half) -> PSUM [128, 1024] f32
        h1sb = {}     # (t, half) -> SBUF [128, 1024] bf16
        h2ps = {}     # (t, c) -> PSUM [128, 512] f32
        h2sb = {}     # (t, c) -> SBUF [128, 512] bf16
        ops_ = {}     # t -> out PSUM [128, 512] f32
        outqs = {}    # quad -> out staging tile [128, 2048] bf16

        def load_quad(q):
            zq = zqp.tile([128, 2048], BF16, tag="zq")
            for a in range(4):
                nc.sync.dma_start(zq[32 * a : 32 * a + 4, :], z4_v[q][a])
            zqs[q] = zq

        def l1(t):
            q, tq = divmod(t, 4)
            h1_ps = ps_h1.tile([128, 2048], F32, tag="h1ps", name="h1ps")
            for a in range(4):
                mm(
                    h1_ps[:, a * 512 : (a + 1) * 512],
                    w1rep[32 * a : 32 * a + 4, :],
                    zqs[q][32 * a : 32 * a + 4, tq * 512 : (tq + 1) * 512],
                    tile_position=(32 * a, 0),
                )
            h1ps[t] = h1_ps

        def evac_h1(t):
            h1_ps = h1ps.pop(t)
            h1_sb = h1p.tile([128, 2048], BF16, tag="h1sb", name="h1sb")
            act(nc.scalar.activation, h1_sb[:], h1_ps[:], AF.Relu)
            h1sb[t] = h1_sb

        def l2_chunk(t, c):
            h1_sb = h1sb[t]
            h2_ps = ps_h2.tile([128, 512], F32, tag="h2ps", name="h2ps")
            mm(h2_ps[:], w2sb[:], h1_sb[:, c * 512 : (c + 1) * 512])
            h2ps[(t, c)] = h2_ps
            if c == 3:
                h1sb.pop(t)

        def evac_h2(t, c):
            h2_ps = h2ps.pop((t, c))
            h2_sb = h2p.tile([128, 512], BF16, tag="h2sb", name="h2sb")
            dve(
                nc.vector.tensor_scalar,
                h2_sb[:], h2_ps[:], b2sb[:], 0.0, op0=ALU.add, op1=ALU.max,
            )
            h2sb[(t, c)] = h2_sb

        def l3(t):
            out_ps = ps_o.tile([128, 512], F32, tag="ops", name="ops")
            for a in range(4):
                mm(
                    out_ps[32 * a : 32 * a + 32, :],
                    w3sb[:],
                    h2sb.pop((t, a))[:],
                    tile_position=(0, 32 * a),
                )
            ops_[t] = out_ps

        def evac_out(t):
            q, tq = divmod(t, 4)
            if tq == 0:
                outqs[q] = oqp.tile([128, 2048], BF16, tag="outq", name="outq")
            out_ps = ops_.pop(t)
            act(
                nc.scalar.activation,
                outqs[q][:, tq * 512 : (tq + 1) * 512], out_ps[:], AF.Copy,
            )
            if tq == 3:
                oq = outqs.pop(q)
                for a in range(4):
                    nc.sync.dma_start(out4_v[q][a], oq[32 * a : 32 * a + NB, :])

        # --- software-pipelined emission ---------------------------------
        for q in range(min(PREFETCH_Q + 1, NQ)):
            load_quad(q)

        for s in range(NT + 2):
            if s < NT and s % 4 == 0:
                qn = s // 4 + PREFETCH_Q + 1
                if qn < NQ:
                    load_quad(qn)
            if s < NT:
                l1(s)
                evac_h1(s)
            if 1 <= s <= NT:
                for c in range(4):
                    l2_chunk(s - 1, c)
                    evac_h2(s - 1, c)
            if 2 <= s <= NT + 1:
                l3(s - 2)
                evac_out(s - 2)

    nc.compile()
    return nc


def _prep_core_z(z_core_rows: np.ndarray, npbf16) -> np.ndarray:
    # [262144, 3] f32 -> [(Q a u), 2048] bf16 with u=3 a ones-row
    zr = z_core_rows.reshape(NQ, 4, 4, 512, 3)          # (Q, tq, a, j, u)
    zr = zr.transpose(0, 2, 4, 1, 3)                    # (Q, a, u, tq, j)
    out = np.ones((NQ, 4, 4, 4, 512), dtype=np.float32)
    out[:, :, :3] = zr
    return np.ascontiguousarray(out.astype(npbf16).reshape(NQ * 16, 2048))


def kernel(z, W1, b1, W2, b2, W3, b3):
    global LAST_RESULTS
    z = np.asarray(z, dtype=np.float32)
    W1 = np.asarray(W1, dtype=np.float32)
    b1 = np.asarray(b1, dtype=np.float32)
    W2 = np.asarray(W2, dtype=np.float32)
    b2 = np.asarray(b2, dtype=np.float32)
    W3 = np.asarray(W3, dtype=np.float32)
    b3 = np.asarray(b3, dtype=np.float32)
    npbf16 = mybir.dt.np(BF16)

    # host-side weight prep (tiny): fold b1 into W1 as 4th input feature
    w1p = np.concatenate([W1, b1.reshape(1, H)], axis=0)  # [4, 128]
    w1rep = np.zeros((128, H), npbf16)
    for a in range(4):
        w1rep[32 * a : 32 * a + 4] = w1p.astype(npbf16)
    w3pad = np.zeros((H, 32), npbf16)
    w3pad[:, :NB] = W3.astype(npbf16)

    z_rows = np.ascontiguousarray(z).reshape(ROWS_TOTAL, 3)
    shards = np.split(z_rows, N_CORES, axis=0)

    common = {
        "w1rep": w1rep,
        "w2": np.ascontiguousarray(W2.astype(npbf16)),
        "w3": w3pad,
        "b2": np.ascontiguousarray(b2.reshape(H, 1)),
    }
    in_maps = [dict(common, z4=_prep_core_z(s, npbf16)) for s in shards]

    nc = _build_nc()
    res = bass_utils.run_bass_kernel_spmd(
        nc,
        in_maps,
        core_ids=list(range(N_CORES)),
        trace=bool(os.environ.get("KBENCH_TRACE")),
    )
    LAST_RESULTS = res

    # host-side un-transpose + b3 + f32 cast
    outs = []
    for i in range(N_CORES):
        o4 = res.results[i]["out4"].astype(np.float32)
        o4 = o4.reshape(NQ, 4, NB, 4, 512)              # (Q, a, u, tq, j)
        o4 = o4.transpose(0, 3, 1, 4, 2)                # (Q, tq, a, j, u)
        outs.append(o4.reshape(ROWS_CORE, NB))
    full = np.concatenate(outs, axis=0) + b3.reshape(1, NB)
    return full.reshape(B, S * NB).astype(np.float32)
